# revision 1
# baseline (speedup 1.0000x reference)
"""Trainium2 Bass kernel for the CustomGATLayer problem.

Strategy: data-parallel over batch B=8 across the 8 NeuronCores (one batch
element per core).  Per core, the masked-softmax GAT attention is computed in
a transposed layout (source node j on partitions, query node i on the free
axis):

    scores[i,j] = LeakyReLU(e_i[i] + e_j[j], 0.2)
    P[j,i]      = exp(scores) * adj[i,j]
                = adj_T[j,i] * max( exp(e_j)[j]*exp(e_i)[i],
                                    exp(.2 e_j)[j]*exp(.2 e_i)[i] )

(the last equality uses monotonicity of exp: exp(max(a,b)) = max(exp a, exp b)
and the rank-1 structure of e_i + e_j).  The O(N^2) work is then only cheap
per-partition-scalar DVE ops, and the exp/transcendental work is O(N).

The softmax denominator is obtained by appending a ones-column to the P@V
stationary operand, so `attended_T = [Wh | 1].T @ P_T` carries the row sums in
its last row; normalization, bias and ReLU happen on the [128,32] output
tiles after a PE transpose back to node-major layout.
"""

import numpy as np

import concourse.bacc as bacc
import concourse.bass as bass
import concourse.mybir as mybir
import concourse.tile as tile
from concourse.bass_utils import run_bass_kernel_spmd
from concourse.masks import make_identity

B, N, D, H, HD = 8, 1024, 256, 8, 32
NT = N // 128  # node tiles per core
WCOLS = H * (HD + 1) + 2 * H  # 264 wh cols (incl. ones col per head) + e_i/e_j cols
F32 = mybir.dt.float32
F16 = mybir.dt.float16
AL = mybir.AluOpType
AF = mybir.ActivationFunctionType

_CACHE: dict = {}
F32R_ = mybir.dt.float32r


def _build_bass():
    nc = bacc.Bacc("TRN2", target_bir_lowering=False, debug=False, num_devices=B)

    xT_d = nc.dram_tensor("xT", [D, N], F16, kind="ExternalInput")
    adjT_d = nc.dram_tensor("adjT", [N, N], F16, kind="ExternalInput")
    wcat_d = nc.dram_tensor("wcat", [D, WCOLS], F16, kind="ExternalInput")
    biasb_d = nc.dram_tensor("biasb", [128, H * HD], F32, kind="ExternalInput")
    out_d = nc.dram_tensor("out", [N, H * HD], F32, kind="ExternalOutput")

    with tile.TileContext(nc) as tc:
        with (
            tc.tile_pool(name="cst", bufs=1) as cst,
            tc.tile_pool(name="wrk", bufs=6) as wrk,
            tc.tile_pool(name="pp", bufs=8) as pp,
            tc.tile_pool(name="pst", bufs=2, space=bass.MemorySpace.PSUM) as pst,
            tc.tile_pool(name="psb", bufs=2, space=bass.MemorySpace.PSUM) as psb,
            tc.tile_pool(name="pvp", bufs=2, space=bass.MemorySpace.PSUM) as pvp,
        ):
            xT = [cst.tile([128, N], F16, tag=f"xT{k}", name=f"xT{k}") for k in range(2)]
            wc = [cst.tile([128, WCOLS], F16, tag=f"wc{k}", name=f"wc{k}") for k in range(2)]
            adjT = [cst.tile([128, N], F16, tag=f"adj{j}", name=f"adj{j}") for j in range(NT)]
            biasb = cst.tile([128, H * HD], F32, tag="biasb", name="biasb_t")
            ident = cst.tile([128, 128], F32, tag="ident", name="ident_t")
            ones_r = cst.tile([1, 128], F16, tag="ones_r", name="ones_r_t")
            ones_f = cst.tile([1, 128], F32, tag="ones_f", name="ones_f_t")
            whb = [cst.tile([128, H * (HD + 1)], F16, tag=f"whb{j}", name=f"whb{j}") for j in range(NT)]
            e_sb = [cst.tile([128, 2 * H], F32, tag=f"e{j}", name=f"e{j}") for j in range(NT)]
            ue1 = [cst.tile([128, H], F32, tag=f"u1{j}", name=f"u1{j}") for j in range(NT)]
            ue2 = [cst.tile([128, H], F32, tag=f"u2{j}", name=f"u2{j}") for j in range(NT)]
            eiT = cst.tile([H, N], F32, tag="eiT", name="eiT_t")
            v1T = cst.tile([H, N], F16, tag="v1T", name="v1T_t")
            v2T = cst.tile([H, N], F16, tag="v2T", name="v2T_t")
            outsb = [cst.tile([128, H * HD], F32, tag=f"o{t}", name=f"o{t}") for t in range(NT)]

            for k in range(2):
                nc.sync.dma_start(xT[k][:], xT_d[k * 128 : (k + 1) * 128, :])
                nc.sync.dma_start(wc[k][:], wcat_d[k * 128 : (k + 1) * 128, :])
            for j in range(NT):
                nc.sync.dma_start(adjT[j][:], adjT_d[j * 128 : (j + 1) * 128, :])
            nc.sync.dma_start(biasb[:], biasb_d[:])
            make_identity(nc, ident[:])
            nc.gpsimd.memset(ones_r[:], 1.0)
            nc.gpsimd.memset(ones_f[:], 1.0)

            # ---- per-head linear transform Wh plus attention logits e ----
            for t in range(NT):
                whp = pst.tile([128, WCOLS], F32, tag="tp", name="whp_t")
                for k in range(2):
                    nc.tensor.matmul(
                        whp[:],
                        xT[k][:, t * 128 : (t + 1) * 128],
                        wc[k][:],
                        start=(k == 0),
                        stop=(k == 1),
                    )
                nc.vector.tensor_copy(whb[t][:], whp[:, 0 : H * (HD + 1)])
                nc.vector.tensor_copy(e_sb[t][:], whp[:, H * (HD + 1) : WCOLS])
                for h in range(H):
                    nc.gpsimd.memset(whb[t][:, h * 33 + 32 : h * 33 + 33], 1.0)
                nc.scalar.activation(ue1[t][:], e_sb[t][:, H : 2 * H], AF.Exp)
                nc.scalar.activation(ue2[t][:], e_sb[t][:, H : 2 * H], AF.Exp, scale=0.2)
                etp = pst.tile([2 * H, 128], F32, tag="tp", name="etp_t")
                nc.tensor.transpose(etp[:], e_sb[t][:], ident[:])
                nc.vector.tensor_copy(eiT[:, t * 128 : (t + 1) * 128], etp[0:H, :])

            nc.scalar.activation(v1T[:], eiT[:], AF.Exp)
            nc.scalar.activation(v2T[:], eiT[:], AF.Exp, scale=0.2)

            # ---- attention per head ----
            # Odd heads: ACT route — Prelu(e_i_bcast + e_j_col, alpha=.2) then
            # Exp, both in the `exp_and_others` act-table set (no table
            # reloads; Copy lives there too).  Even heads: DVE route — rank-1
            # products of exp'd factors + tt max.  tt-class ops (max/mask)
            # split DVE/GPSIMD to balance engines.
            F32R = mybir.dt.float32r
            tt_count = 0
            TT_GPS = (32, 64)  # 31 of 96 tt ops on gpsimd

            def tt_engine():
                nonlocal tt_count
                k = (tt_count * TT_GPS[0]) % TT_GPS[1] < TT_GPS[0]
                tt_count += 1
                return nc.gpsimd if k else nc.vector

            for h in range(H):
                act_route = h % 2 == 1
                pvt = pvp.tile([HD + 1, N], F32, tag="pv", name="pvt_t")
                if act_route:
                    # broadcast raw e_i row across partitions (gpsimd, SBUF)
                    B1 = wrk.tile([128, N], F32, tag="B1", name="B1_t", bufs=2)
                    eis = wrk.tile([1, N], F32, tag="eis", name="eis_t", bufs=2)
                    nc.sync.dma_start(eis[:], eiT[h : h + 1, :])
                    nc.gpsimd.partition_broadcast(B1[:], eis[:])
                    for j in range(NT):
                        lr = wrk.tile([128, N], F32, tag="lr", name="lr_t", bufs=8)
                        Pp = wrk.tile([128, N], F16, tag="Pp", name="Pp_t", bufs=8)
                        P = pp.tile([128, N], F16, tag="P", name="P_t")
                        nc.scalar.activation(
                            lr[:],
                            B1[:],
                            AF.Prelu,
                            bias=e_sb[j][:, H + h : H + h + 1],
                            alpha=0.2,
                        )
                        nc.scalar.activation(Pp[:], lr[:], AF.Exp)
                        tt_engine().tensor_tensor(P[:], Pp[:], adjT[j][:], AL.mult)
                        for half in range(2):
                            sl = slice(half * 512, (half + 1) * 512)
                            nc.tensor.matmul(
                                pvt[:, sl],
                                whb[j][:, h * 33 : (h + 1) * 33],
                                P[:, sl],
                                start=(j == 0),
                                stop=(j == NT - 1),
                            )
                else:
                    V1 = wrk.tile([128, N], F16, tag="V1", name="V1_t", bufs=2)
                    V2 = wrk.tile([128, N], F16, tag="V2", name="V2_t", bufs=2)
                    v1s = wrk.tile([1, N], F16, tag="v1s", name="v1s_t", bufs=2)
                    v2s = wrk.tile([1, N], F16, tag="v2s", name="v2s_t", bufs=2)
                    # stage head row at partition 0 (PE rhs needs base partition 0)
                    nc.sync.dma_start(v1s[:], v1T[h : h + 1, :])
                    nc.sync.dma_start(v2s[:], v2T[h : h + 1, :])
                    for Vt, vs in ((V1, v1s), (V2, v2s)):
                        for half in range(2):
                            sl = slice(half * 512, (half + 1) * 512)
                            bp = psb.tile([128, 512], F32, tag="bc", name="bp_t")
                            nc.tensor.matmul(bp[:], ones_r[:], vs[0:1, sl])
                            nc.scalar.copy(Vt[:, sl], bp[:])
                    for j in range(NT):
                        E1 = wrk.tile([128, N], F16, tag="E1", name="E1_t", bufs=8)
                        E2 = wrk.tile([128, N], F16, tag="E2", name="E2_t", bufs=8)
                        Pp = wrk.tile([128, N], F16, tag="Pp", name="Pp_t", bufs=8)
                        P = pp.tile([128, N], F16, tag="P", name="P_t")
                        nc.vector.tensor_scalar(
                            E1[:], V1[:], ue1[j][:, h : h + 1], None, AL.mult
                        )
                        nc.vector.tensor_scalar(
                            E2[:], V2[:], ue2[j][:, h : h + 1], None, AL.mult
                        )
                        nc.vector.tensor_tensor(Pp[:], E1[:], E2[:], AL.max)
                        tt_engine().tensor_tensor(P[:], Pp[:], adjT[j][:], AL.mult)
                        for half in range(2):
                            sl = slice(half * 512, (half + 1) * 512)
                            nc.tensor.matmul(
                                pvt[:, sl],
                                whb[j][:, h * 33 : (h + 1) * 33],
                                P[:, sl],
                                start=(j == 0),
                                stop=(j == NT - 1),
                            )

                pvs = wrk.tile([HD + 1, N], F32, tag="pvs", name="pvs_t", bufs=2)
                nc.scalar.copy(pvs[:], pvt[:])
                for t in range(NT):
                    atp = pst.tile([128, HD + 1], F32, tag="tp", name="atp_t")
                    nc.tensor.transpose(
                        atp[:], pvs[:, t * 128 : (t + 1) * 128], ident[0:33, 0:33]
                    )
                    rc = wrk.tile([128, 1], F32, tag="rc", name="rc_t", bufs=4)
                    nc.vector.reciprocal(rc[:], atp[:, HD : HD + 1])
                    tmp = wrk.tile([128, HD], F32, tag="tmp", name="tmp_t", bufs=4)
                    nc.vector.scalar_tensor_tensor(
                        tmp[:],
                        atp[:, 0:HD],
                        rc[:],
                        biasb[:, h * HD : (h + 1) * HD],
                        AL.mult,
                        AL.add,
                    )
                    nc.vector.tensor_scalar(
                        outsb[t][:, h * HD : (h + 1) * HD], tmp[:], 0.0, None, AL.max
                    )

            for t in range(NT):
                nc.sync.dma_start(out_d[t * 128 : (t + 1) * 128, :], outsb[t][:])

    nc.compile()
    return nc


def get_nc():
    if "nc" not in _CACHE:
        _CACHE["nc"] = _build_bass()
    return _CACHE["nc"]


def host_prep(node_features, adjacency, W, a, bias):
    node_features = np.asarray(node_features, dtype=np.float32)
    adjacency = np.asarray(adjacency)
    W = np.asarray(W, dtype=np.float32)
    a = np.asarray(a, dtype=np.float32)
    bias = np.asarray(bias, dtype=np.float32)

    wcat = np.zeros((D, WCOLS), np.float32)
    for h in range(H):
        wcat[:, h * 33 : h * 33 + HD] = W[h]
        wcat[:, H * (HD + 1) + h] = W[h] @ a[h, :HD]  # e_i term
        wcat[:, H * (HD + 1) + H + h] = W[h] @ a[h, HD:]  # e_j term
    biasb = np.broadcast_to(bias, (128, H * HD)).copy()

    in_maps = []
    for b in range(B):
        in_maps.append(
            {
                "xT": np.ascontiguousarray(node_features[b].T).astype(np.float16),
                "adjT": np.ascontiguousarray(adjacency[b].T).astype(np.float16),
                "wcat": wcat.astype(np.float16),
                "biasb": biasb,
            }
        )
    return in_maps


def kernel(node_features, adjacency, W, a, bias):
    nc = get_nc()
    in_maps = host_prep(node_features, adjacency, W, a, bias)
    res = run_bass_kernel_spmd(nc, in_maps, core_ids=list(range(B)))
    return np.stack([res.results[b]["out"] for b in range(B)], axis=0)

